# revision 1
# baseline (speedup 1.0000x reference)
"""Self-contained Trainium2 Bass kernel for the 3-layer stacked GRU encoder
(nn_NoisyGRUSeq2SeqWithFeatures).

Strategy: 8-way MODEL-parallel (output-channel sharding) with fp16
weights/activations/state (fp32 PSUM accumulation) so all weights stay
SBUF-resident across the 64-step recurrence.  fp16 runs the PE at
1 cycle/row at every tile size (f32r pays 4x under 256-wide outputs)
and halves all gather/staging bytes; end-to-end rel err ~6.6e-3 vs the
2e-2 tolerance.  Full batch (B=128) replicated per core so every matmul
has a 128-wide stationary operand.  Per wave a layer-skewed schedule
(L0 at t, L1 at t-1, L2 at t-2) fuses the six per-step AllGathers into
two; candidate bias/x-part matmuls go to per-layer PSUM banks (PE
accumulation brackets must stay contiguous per bank) and are emitted
ahead of the rh-parts so they execute inside the AG(rh) collective wait.
On this runtime collectives cost a flat ~25-32us of serial queue
occupancy each and never pipeline, so the wave floor is the two
collectives (~65us/wave, ~4.3ms total); the 24us/wave of compute is
fully hidden.  Variable sequence lengths: +30 added to the pre-sigmoid
gate logits of finished samples (z -> 1 freezes h).  The dead h-gather
of the final wave is skipped.
"""

import numpy as np

SIZES = (512, 1024, 2048)
EMB = 32
VOC = 40
LATENT = 512
B, S = 128, 64
NCORES = 8

# per-core output slice sizes per layer
SL = tuple(sz // NCORES for sz in SIZES)  # (64, 128, 256)
HLOC = sum(SL)  # 448 columns of per-core h state
import os as _os

F32 = None  # set after mybir import
SKIP_COLL = bool(int(_os.environ.get("SKIP_COLL", "0")))  # debug: skip collectives


def _sbufize(w: np.ndarray, tp: int = 128) -> np.ndarray:
    """[nk*tp, ncols] row-tiled weight -> SBUF layout [tp, nk*ncols]."""
    nk = w.shape[0] // tp
    assert w.shape[0] == nk * tp
    return (
        np.ascontiguousarray(w.reshape(nk, tp, w.shape[1]).transpose(1, 0, 2))
        .reshape(tp, nk * w.shape[1])
        .astype(np.float16)
    )


def prepack(inputs: dict) -> list[dict]:
    """Build per-core in_maps from the full (unsharded) problem inputs."""
    seqs = np.asarray(inputs["input_seqs"])
    lens = np.asarray(inputs["input_lens"])
    emb = np.asarray(inputs["emb"], np.float32)
    Kg = [np.asarray(inputs[f"Kg{l}"], np.float32) for l in range(3)]
    bg = [np.asarray(inputs[f"bg{l}"], np.float32) for l in range(3)]
    Kc = [np.asarray(inputs[f"Kc{l}"], np.float32) for l in range(3)]
    bc = [np.asarray(inputs[f"bc{l}"], np.float32) for l in range(3)]
    W_enc = np.asarray(inputs["W_enc"], np.float32)
    b_enc = np.asarray(inputs["b_enc"], np.float32)

    # shared tensors
    x_emb = emb[seqs]  # [B,S,EMB]
    xembT = np.zeros((EMB + 1, S * B), np.float16)
    for t in range(S):
        xembT[:EMB, t * B : (t + 1) * B] = x_emb[:, t, :].T
    xembT[EMB, :] = 1.0
    maskb = np.zeros((B, S), np.float32)
    for t in range(S):
        maskb[:, t] = np.where(t < lens, 0.0, 30.0)
    ident = np.eye(128, dtype=np.float16)
    ones_row = np.ones((1, B), np.float16)

    xin = (EMB, SIZES[0], SIZES[1])  # x-input width per layer
    in_maps = []
    for i in range(NCORES):
        m = {
            "xembT": xembT,
            "maskb": maskb,
            "ident": ident,
            "ones_row": ones_row,
        }
        ccols = {}
        for l in range(3):
            sl, cout = SL[l], SIZES[l]
            rcols = sl * i + np.arange(sl)
            gcols = np.concatenate([rcols, cout + rcols])
            ccols[l] = rcols
            cin = xin[l]
            if l == 0:
                m["kg0x"] = np.vstack([Kg[0][:cin, gcols], bg[0][gcols][None, :]]).astype(np.float16)
                m["kc0x"] = np.vstack([Kc[0][:cin, ccols[0]], bc[0][ccols[0]][None, :]]).astype(np.float16)
            else:
                tpx = 64 if l == 1 else 128  # L1 x-input is h0 (64-row rank chunks)
                m[f"kg{l}x"] = _sbufize(Kg[l][:cin, gcols], tpx)
                m[f"kc{l}x"] = _sbufize(Kc[l][:cin, ccols[l]], tpx)
                m[f"bg{l}row"] = bg[l][gcols][None, :].astype(np.float16)
                m[f"bc{l}row"] = bc[l][ccols[l]][None, :].astype(np.float16)
            tp = 64 if l == 0 else 128
            m[f"kg{l}h"] = _sbufize(Kg[l][cin:, gcols], tp)
            m[f"kc{l}h"] = _sbufize(Kc[l][cin:, ccols[l]], tp)
        # W_enc rows for this core's h slices, in h_loc order, padded to 512
        wrows = np.concatenate(
            [
                512 + 128 * i + np.arange(128),
                1536 + 256 * i + np.arange(256),
                64 * i + np.arange(64),
            ]
        )
        wenc = np.zeros((512, LATENT), np.float16)
        wenc[:HLOC] = W_enc[wrows]
        m["wenc"] = _sbufize(wenc)
        m["benc_row"] = (b_enc / NCORES)[None, :].astype(np.float16)
        in_maps.append(m)
    return in_maps


def build(n_waves=S + 2, repeat=1):
    import concourse.bass as bass
    import concourse.bacc as bacc
    import concourse.tile as tile
    import concourse.mybir as mybir

    f32 = mybir.dt.float32
    AF = mybir.ActivationFunctionType
    nc = bacc.Bacc("TRN2", target_bir_lowering=False, debug=False, num_devices=NCORES)

    # h_loc column layout: [h1 (0:128) | h2 (128:384) | h0 (384:448)]
    # rank chunk (transposed, padded to 512 rows): same order + 64 pad rows
    dshapes = {
        "xembT": [EMB + 1, S * B],
        "maskb": [B, S],
        "ident": [128, 128],
        "ones_row": [1, B],
        "kg0x": [EMB + 1, 2 * SL[0]],
        "kc0x": [EMB + 1, SL[0]],
        "kg0h": [64, 8 * 2 * SL[0]],
        "kc0h": [64, 8 * SL[0]],
        "kg1x": [64, 8 * 2 * SL[1]],
        "kc1x": [64, 8 * SL[1]],
        "kg1h": [128, 8 * 2 * SL[1]],
        "kc1h": [128, 8 * SL[1]],
        "bg1row": [1, 2 * SL[1]],
        "bc1row": [1, SL[1]],
        "kg2x": [128, 8 * 2 * SL[2]],
        "kc2x": [128, 8 * SL[2]],
        "kg2h": [128, 16 * 2 * SL[2]],
        "kc2h": [128, 16 * SL[2]],
        "bg2row": [1, 2 * SL[2]],
        "bc2row": [1, SL[2]],
        "wenc": [128, 4 * LATENT],
        "benc_row": [1, LATENT],
    }
    f32r = mybir.dt.float32r
    f16 = mybir.dt.float16
    dram = {
        k: nc.dram_tensor(k, v, f32 if k == "maskb" else f16, kind="ExternalInput")
        for k, v in dshapes.items()
    }
    out_d = nc.dram_tensor("out", [B, LATENT], f32, kind="ExternalOutput")

    with tile.TileContext(nc) as tc:
        with (
            tc.tile_pool(name="wts", bufs=1) as wp,
            tc.tile_pool(name="acts", bufs=1) as ap,
            tc.tile_pool(name="hbuf", bufs=1) as hp,
            tc.tile_pool(name="stg", bufs=2) as sp,
            tc.tile_pool(name="gates", bufs=1) as gp,
            tc.tile_pool(name="psg", bufs=1, space="PSUM") as psg,
            tc.tile_pool(name="psc", bufs=1, space="PSUM") as psc,
            tc.tile_pool(name="pst", bufs=2, space="PSUM") as pst,
            tc.tile_pool(name="dram", bufs=2, space="DRAM") as dp,
        ):
            w = {}
            for k in dshapes:
                t = wp.tile(dshapes[k], f32 if k == "maskb" else f16, name=f"w_{k}")
                nc.sync.dma_start(t[:], dram[k][:])
                w[k] = t

            def wt(name, ncols, j, tp=128):
                return w[name][0:tp, j * ncols : (j + 1) * ncols]

            f32r = mybir.dt.float32r

            def mm(out, lhsT, rhs, **kw):
                nc.tensor.matmul(out, lhsT, rhs, **kw)

            h_loc = ap.tile([B, HLOC], f16, name="h_loc")
            rh_loc = ap.tile([B, HLOC], f16, name="rh_loc")
            nc.vector.memset(h_loc[:].bitcast(f32), 0.0)
            nc.vector.memset(rh_loc[:].bitcast(f32), 0.0)

            def fresh_hT(tag):
                return hp.tile([128, NCORES * 4 * B], f16, name=f"{tag}T", tag=f"{tag}T")

            def RK(t, r, k):
                return t[:, (4 * r + k) * B : (4 * r + k + 1) * B]

            def RK64(t, r):
                return t[0:64, (4 * r + 3) * B : (4 * r + 3) * B + B]

            hT = fresh_hT("h")
            nc.vector.memset(hT[:].bitcast(f32), 0.0)

            def stage_and_gather(src_loc, tag, wv):
                pt = pst.tile([128, 512], f16, name=f"pt_{tag}{wv}", tag="pt")
                for k in range(4):
                    csz = 128 if k < 3 else HLOC - 384
                    nc.tensor.transpose(
                        pt[:csz, k * 128 : k * 128 + 128],
                        src_loc[:, k * 128 : k * 128 + csz],
                        w["ident"][:],
                    )
                stg = sp.tile([128, 512], f16, name=f"stg_{tag}{wv}", tag=f"stg{tag}")
                nc.vector.tensor_copy(stg[:], pt[:])
                agin = dp.tile([128, 512], f16, name=f"agin_{tag}{wv}", tag=f"agin{tag}")
                agout = dp.tile(
                    [NCORES * 128, 512], f16,
                    name=f"agout_{tag}{wv}", tag=f"agout{tag}", addr_space="Shared",
                )
                nc.sync.dma_start(agin[:], stg[:])
                if SKIP_COLL:
                    nc.sync.dma_start(agout[0:128, :], agin[:])
                else:
                    nc.gpsimd.collective_compute(
                        "AllGather",
                        mybir.AluOpType.bypass,
                        replica_groups=[list(range(NCORES))],
                        ins=[agin[:]],
                        outs=[agout[:]],
                    )
                gT = fresh_hT(tag)
                for r in range(NCORES):
                    nc.sync.dma_start(
                        gT[:, r * 512 : (r + 1) * 512],
                        agout[r * 128 : (r + 1) * 128, :],
                    )
                return gT

            for rep in range(repeat):
              for wv0 in range(n_waves):
                wv = rep * n_waves + wv0
                t0, t1, t2 = wv0, wv0 - 1, wv0 - 2

                # ---------------- gates ----------------
                pg2 = psg.tile([B, 2 * SL[2]], f32, name=f"pg2_{wv}", tag="pg2")
                pg01 = psg.tile([B, 2 * (SL[0] + SL[1])], f32, name=f"pg01_{wv}", tag="pg01")
                if 0 <= t2 < S:
                    mm(pg2[:], w["ones_row"][:], w["bg2row"][:], start=True, stop=False)
                    for r in range(NCORES):
                        mm(pg2[:], RK(hT, r, 0), wt("kg2x", 2 * SL[2], r),
                                         start=False, stop=False)
                    for r in range(NCORES):
                        for a in range(2):
                            mm(pg2[:], RK(hT, r, 1 + a), wt("kg2h", 2 * SL[2], 2 * r + a),
                                             start=False, stop=(r == NCORES - 1 and a == 1))
                if 0 <= t1 < S:
                    mm(pg01[:, 128:384], w["ones_row"][:], w["bg1row"][:], start=True, stop=False)
                    for r in range(NCORES):
                        mm(pg01[:, 128:384], RK64(hT, r), wt("kg1x", 2 * SL[1], r, 64),
                                         start=False, stop=False)
                    for r in range(NCORES):
                        mm(pg01[:, 128:384], RK(hT, r, 0), wt("kg1h", 2 * SL[1], r),
                                         start=False, stop=(r == NCORES - 1))
                if t0 < S:
                    mm(pg01[:, 0:128], w["xembT"][:, t0 * B : (t0 + 1) * B], w["kg0x"][:], start=True, stop=False)
                    for r in range(NCORES):
                        mm(pg01[:, 0:128], RK64(hT, r), wt("kg0h", 2 * SL[0], r, 64),
                                         start=False, stop=(r == NCORES - 1))

                # ---------------- sigmoid + r*h ----------------
                g2sb = gp.tile([B, 2 * SL[2]], f16, name=f"g2sb_{wv}", tag="g2sb")
                g01sb = gp.tile([B, 2 * (SL[0] + SL[1])], f16, name=f"g01sb_{wv}", tag="g01sb")
                if 0 <= t2 < S:
                    nc.scalar.activation(g2sb[:], pg2[:], AF.Sigmoid, bias=w["maskb"][:, t2 : t2 + 1])
                    nc.vector.tensor_mul(rh_loc[:, 128:384], g2sb[:, 0 : SL[2]], h_loc[:, 128:384])
                if 0 <= t1 < S:
                    nc.scalar.activation(g01sb[:, 128:384], pg01[:, 128:384], AF.Sigmoid,
                                         bias=w["maskb"][:, t1 : t1 + 1])
                    nc.vector.tensor_mul(rh_loc[:, 0:128], g01sb[:, 128 : 128 + SL[1]], h_loc[:, 0:128])
                if t0 < S:
                    nc.scalar.activation(g01sb[:, 0:128], pg01[:, 0:128], AF.Sigmoid,
                                         bias=w["maskb"][:, t0 : t0 + 1])
                    nc.vector.tensor_mul(rh_loc[:, 384:HLOC], g01sb[:, 0 : SL[0]], h_loc[:, 384:HLOC])

                # ---------------- AG(rh) ----------------
                rhT = stage_and_gather(rh_loc, "r", wv)

                # ---------------- candidates ----------------
                # Each layer's candidate gets its own PSUM tile (own bank) so
                # accumulation brackets stay per-bank while x-parts (ready:
                # depend only on hT/xembT) are emitted before rh-parts (stall
                # on the AG(rh) fan-in) -> x-parts fill the collective wait.
                pc1 = psc.tile([B, SL[1]], f32, name=f"pc1_{wv}", tag="pc1")
                pc2 = psc.tile([B, SL[2]], f32, name=f"pc2_{wv}", tag="pc2")
                pc0 = psc.tile([B, SL[0]], f32, name=f"pc0_{wv}", tag="pc0")
                if 0 <= t1 < S:
                    mm(pc1[:], w["ones_row"][:], w["bc1row"][:], start=True, stop=False)
                    for r in range(NCORES):
                        mm(pc1[:], RK64(hT, r), wt("kc1x", SL[1], r, 64),
                                         start=False, stop=False)
                if 0 <= t2 < S:
                    mm(pc2[:], w["ones_row"][:], w["bc2row"][:], start=True, stop=False)
                    for r in range(NCORES):
                        mm(pc2[:], RK(hT, r, 0), wt("kc2x", SL[2], r),
                                         start=False, stop=False)
                if t0 < S:
                    mm(pc0[:], w["xembT"][:, t0 * B : (t0 + 1) * B], w["kc0x"][:], start=True, stop=False)
                if 0 <= t1 < S:
                    for r in range(NCORES):
                        mm(pc1[:], RK(rhT, r, 0), wt("kc1h", SL[1], r),
                                         start=False, stop=(r == NCORES - 1))
                if 0 <= t2 < S:
                    for r in range(NCORES):
                        for a in range(2):
                            mm(pc2[:], RK(rhT, r, 1 + a), wt("kc2h", SL[2], 2 * r + a),
                                             start=False, stop=(r == NCORES - 1 and a == 1))
                if t0 < S:
                    for r in range(NCORES):
                        mm(pc0[:], RK64(rhT, r), wt("kc0h", SL[0], r, 64),
                                         start=False, stop=(r == NCORES - 1))

                # ---------------- tanh + h_new ----------------
                csb = gp.tile([B, HLOC], f16, name=f"csb_{wv}", tag="csb")
                if 0 <= t1 < S:
                    nc.scalar.activation(csb[:, 0:128], pc1[:], AF.Tanh)
                if 0 <= t2 < S:
                    nc.scalar.activation(csb[:, 128:384], pc2[:], AF.Tanh)
                if t0 < S:
                    nc.scalar.activation(csb[:, 384:HLOC], pc0[:], AF.Tanh)
                tmp2 = gp.tile([B, SL[2]], f16, name=f"tmp2_{wv}", tag="tmp2")
                zsl = {
                    0: (g01sb, SL[0], 384, HLOC),
                    1: (g01sb, 256, 0, 128),
                    2: (g2sb, SL[2], 128, 384),
                }
                for l, tl in ((0, t0), (1, t1), (2, t2)):
                    if tl < 0 or tl >= S:
                        continue
                    gt, zoff, a, b2 = zsl[l]
                    sw = b2 - a
                    nc.vector.tensor_sub(tmp2[:, :sw], h_loc[:, a:b2], csb[:, a:b2])
                    nc.vector.tensor_mul(tmp2[:, :sw], gt[:, zoff : zoff + sw], tmp2[:, :sw])
                    nc.vector.tensor_add(h_loc[:, a:b2], tmp2[:, :sw], csb[:, a:b2])

                # ---------------- AG(h) ----------------
                # last wave's h-gather has no consumer -> skip it
                if not (rep == repeat - 1 and wv0 == n_waves - 1):
                    hT = stage_and_gather(h_loc, "h", wv)

            # ---------------- final projection ----------------
            ptf = pst.tile([128, 512], f16, name="ptf", tag="pt")
            for k in range(4):
                csz = 128 if k < 3 else HLOC - 384
                nc.tensor.transpose(
                    ptf[:csz, k * 128 : k * 128 + 128],
                    h_loc[:, k * 128 : k * 128 + csz],
                    w["ident"][:],
                )
            hsf = sp.tile([128, 512], f16, name="hsf", tag="stgh")
            nc.vector.tensor_copy(hsf[:], ptf[:])
            nc.vector.memset(hsf[64:128, 384:512].bitcast(f32), 0.0)
            pz = psg.tile([B, LATENT], f32, name="pz", tag="pg2")
            mm(pz[:], w["ones_row"][:], w["benc_row"][:], start=True, stop=False)
            for k in range(4):
                mm(pz[:], hsf[:, k * 128 : (k + 1) * 128],
                                 wt("wenc", LATENT, k), start=False, stop=(k == 3))
            zsb = gp.tile([B, LATENT], f32, name="zsb", tag="g2sb")
            nc.vector.tensor_copy(zsb[:], pz[:])
            arin = dp.tile([B, LATENT], f32, name="arin")
            arout = dp.tile([B, LATENT], f32, name="arout", addr_space="Shared")
            nc.sync.dma_start(arin[:], zsb[:])
            if SKIP_COLL:
                nc.sync.dma_start(arout[:], arin[:])
            else:
                nc.gpsimd.collective_compute(
                    "AllReduce",
                    mybir.AluOpType.add,
                    replica_groups=[list(range(NCORES))],
                    ins=[arin[:]],
                    outs=[arout[:]],
                )
            zfull = gp.tile([B, LATENT], f32, name="zfull", tag="csb")
            nc.sync.dma_start(zfull[:], arout[:])
            ofin = gp.tile([B, LATENT], f32, name="ofin", tag="g01sb")
            nc.scalar.activation(ofin[:], zfull[:], AF.Tanh)
            nc.sync.dma_start(out_d[:], ofin[:])

    nc.compile()
    return nc


_NC_CACHE = {}


def kernel(**inputs) -> np.ndarray:
    from concourse import bass_utils

    if "nc" not in _NC_CACHE:
        _NC_CACHE["nc"] = build()
    nc = _NC_CACHE["nc"]
    in_maps = prepack(inputs)
    res = bass_utils.run_bass_kernel_spmd(nc, in_maps, core_ids=list(range(NCORES)))
    return np.asarray(res.results[0]["out"], np.float32)



# revision 2
# speedup vs baseline: 1.0505x; 1.0505x over previous
"""Self-contained Trainium2 Bass kernel for the 3-layer stacked GRU encoder
(nn_NoisyGRUSeq2SeqWithFeatures), v3.

Hybrid replication/model-parallel: L0 (512) and L1 (1024) are REPLICATED on
every core (their full-step compute is only ~12us/wave at measured PE speed
of ~135ns per 512-col fp16 matmul), so they need NO collectives at all.
Only L2 (2048) is 8-way output-channel sharded, keeping the two per-wave
AllGathers (rh2, h2) at a 512KB output payload (measured AG cost ~11.6us/MB
out + small constant; it was ~1MB x2 in the baseline).  The replicated
L0/L1 chains have no dependency on the in-flight collectives, so they fill
the AG windows; per-wave exposed work drops to the L2 gate/cand chains.

Everything is fp16 with f32 PSUM accumulation.  Ragged lengths use the
baseline's +30 pre-sigmoid mask bias (z->1 freezes h).  Layer skew: L0 at
t, L1 at t-1, L2 at t-2; 66 waves.  Final projection: each core computes
[h1; h2_slice; h0] @ W_enc rows with the replicated parts pre-divided by 8,
then one AllReduce.
"""

import numpy as np

SIZES = (512, 1024, 2048)
EMB = 32
VOC = 40
LATENT = 512
B, S = 128, 64
NCORES = 8

SL2 = SIZES[2] // NCORES  # 256
import os as _os

SKIP_COLL = bool(int(_os.environ.get("SKIP_COLL", "0")))
SKIP_UNPACK = bool(int(_os.environ.get("SKIP_UNPACK", "0")))
SKIP_L01 = bool(int(_os.environ.get("SKIP_L01", "0")))
SKIP_L2MM = bool(int(_os.environ.get("SKIP_L2MM", "0")))


def _sbufize(w: np.ndarray, tp: int = 128) -> np.ndarray:
    """[nk*tp, ncols] row-tiled weight -> SBUF layout [tp, nk*ncols]."""
    nk = w.shape[0] // tp
    assert w.shape[0] == nk * tp
    return (
        np.ascontiguousarray(w.reshape(nk, tp, w.shape[1]).transpose(1, 0, 2))
        .reshape(tp, nk * w.shape[1])
        .astype(np.float16)
    )


def prepack(inputs: dict) -> list[dict]:
    seqs = np.asarray(inputs["input_seqs"])
    lens = np.asarray(inputs["input_lens"])
    emb = np.asarray(inputs["emb"], np.float32)
    Kg = [np.asarray(inputs[f"Kg{l}"], np.float32) for l in range(3)]
    bg = [np.asarray(inputs[f"bg{l}"], np.float32) for l in range(3)]
    Kc = [np.asarray(inputs[f"Kc{l}"], np.float32) for l in range(3)]
    bc = [np.asarray(inputs[f"bc{l}"], np.float32) for l in range(3)]
    W_enc = np.asarray(inputs["W_enc"], np.float32)
    b_enc = np.asarray(inputs["b_enc"], np.float32)

    x_emb = emb[seqs]  # [B,S,EMB]
    xembB = np.ascontiguousarray(x_emb.reshape(B, S * EMB)).astype(np.float16)
    maskb = np.zeros((B, S), np.float32)
    for t in range(S):
        maskb[:, t] = np.where(t < lens, 0.0, 30.0)
    ident = np.eye(128, dtype=np.float16)
    ones_row = np.ones((1, B), np.float16)

    # replicated L0/L1 weights (identical on every core)
    shared = {
        "xembB": xembB,
        "maskb": maskb,
        "ident": ident,
        "ones_row": ones_row,
        # L0: x rows = emb(32)+bias; h rows = 512
        "kg0x": np.vstack([Kg[0][:EMB], bg[0][None, :]]).astype(np.float16),  # [33,1024]
        "kc0x": np.vstack([Kc[0][:EMB], bc[0][None, :]]).astype(np.float16),  # [33,512]
        "kg0h": _sbufize(Kg[0][EMB:], 128),    # [128, 4*1024]
        "kc0h": _sbufize(Kc[0][EMB:], 128),    # [128, 4*512]
        # L1: x rows = h0(512); h rows = 1024
        "kg1x": _sbufize(Kg[1][:512], 128),    # [128, 4*2048]
        "kc1x": _sbufize(Kc[1][:512], 128),    # [128, 4*1024]
        "kg1h": _sbufize(Kg[1][512:], 128),    # [128, 8*2048]
        "kc1h": _sbufize(Kc[1][512:], 128),    # [128, 8*1024]
        "bg1row": bg[1][None, :].astype(np.float16),  # [1,2048]
        "bc1row": bc[1][None, :].astype(np.float16),  # [1,1024]
    }

    in_maps = []
    for i in range(NCORES):
        m = dict(shared)
        cols2 = SL2 * i + np.arange(SL2)
        gcols2 = np.concatenate([cols2, SIZES[2] + cols2])
        # L2 sharded: x rows = h1(1024); h rows = 2048
        m["kg2x"] = _sbufize(Kg[2][:1024, gcols2], 128)   # [128, 8*512]
        m["kc2x"] = _sbufize(Kc[2][:1024, cols2], 128)    # [128, 8*256]
        m["kg2h"] = _sbufize(Kg[2][1024:, gcols2], 128)   # [128, 16*512]
        m["kc2h"] = _sbufize(Kc[2][1024:, cols2], 128)    # [128, 16*256]
        m["bg2row"] = bg[2][gcols2][None, :].astype(np.float16)
        m["bc2row"] = bc[2][cols2][None, :].astype(np.float16)
        # W_enc rows: [h1 full (1024, /8) ; h2 slice (256) ; h0 full (512, /8)]
        wenc = np.concatenate(
            [
                W_enc[512:1536] / NCORES,
                W_enc[1536 + SL2 * i : 1536 + SL2 * (i + 1)],
                W_enc[0:512] / NCORES,
            ]
        ).astype(np.float16)  # [1792, 512]
        m["wenc"] = _sbufize(wenc, 128)  # [128, 14*512]
        m["benc_row"] = (b_enc / NCORES)[None, :].astype(np.float16)
        in_maps.append(m)
    return in_maps


def build(n_waves=S + 2, repeat=1):
    import concourse.bass as bass
    import concourse.bacc as bacc
    import concourse.tile as tile
    import concourse.mybir as mybir

    f32 = mybir.dt.float32
    f16 = mybir.dt.float16
    AF = mybir.ActivationFunctionType
    nc = bacc.Bacc("TRN2", target_bir_lowering=False, debug=False, num_devices=NCORES)

    dshapes = {
        "xembB": [B, S * EMB],
        "maskb": [B, S],
        "ident": [128, 128],
        "ones_row": [1, B],
        "kg0x": [EMB + 1, 1024],
        "kc0x": [EMB + 1, 512],
        "kg0h": [128, 4 * 1024],
        "kc0h": [128, 4 * 512],
        "kg1x": [128, 4 * 2048],
        "kc1x": [128, 4 * 1024],
        "kg1h": [128, 8 * 2048],
        "kc1h": [128, 8 * 1024],
        "bg1row": [1, 2048],
        "bc1row": [1, 1024],
        "kg2x": [128, 8 * 2 * SL2],
        "kc2x": [128, 8 * SL2],
        "kg2h": [128, 16 * 2 * SL2],
        "kc2h": [128, 16 * SL2],
        "bg2row": [1, 2 * SL2],
        "bc2row": [1, SL2],
        "wenc": [128, 14 * LATENT],
        "benc_row": [1, LATENT],
    }
    dram = {
        k: nc.dram_tensor(k, v, f32 if k == "maskb" else f16, kind="ExternalInput")
        for k, v in dshapes.items()
    }
    out_d = nc.dram_tensor("out", [B, LATENT], f32, kind="ExternalOutput")

    with tile.TileContext(nc) as tc:
        with (
            tc.tile_pool(name="wts", bufs=1) as wp,
            tc.tile_pool(name="acts", bufs=1) as ap,
            tc.tile_pool(name="tps", bufs=2) as tp_pool,
            tc.tile_pool(name="tpr", bufs=1) as tpr_pool,
            tc.tile_pool(name="hbuf", bufs=2) as hp,
            tc.tile_pool(name="stg", bufs=2) as sp,
            tc.tile_pool(name="gates", bufs=1) as gp,
            tc.tile_pool(name="ps2", bufs=1, space="PSUM") as ps2,
            tc.tile_pool(name="psb", bufs=2, space="PSUM") as psb,
            tc.tile_pool(name="pst", bufs=2, space="PSUM") as pst,
            tc.tile_pool(name="dram", bufs=2, space="DRAM") as dp,
        ):
            w = {}
            for k in dshapes:
                t = wp.tile(dshapes[k], f32 if k == "maskb" else f16, name=f"w_{k}")
                nc.sync.dma_start(t[:], dram[k][:])
                w[k] = t

            def wt(name, ncols, j, tp=128):
                return w[name][0:tp, j * ncols : (j + 1) * ncols]

            def mm(out, lhsT, rhs, **kw):
                nc.tensor.matmul(out, lhsT, rhs, **kw)

            # ---- persistent state ----
            h0 = ap.tile([B, 512], f16, name="h0")
            h1 = ap.tile([B, 1024], f16, name="h1")
            h2 = ap.tile([B, SL2], f16, name="h2")      # local slice
            rh0 = ap.tile([B, 512], f16, name="rh0")
            rh1 = ap.tile([B, 1024], f16, name="rh1")
            rh2 = ap.tile([B, SL2], f16, name="rh2")
            for t in (h0, h1, h2, rh0, rh1, rh2):
                nc.vector.memset(t[:].bitcast(f32), 0.0)

            # transposed copies (lhsT), double-buffered across waves
            def fresh(tag, cols):
                return tp_pool.tile([128, cols], f16, name=tag, tag=tag)

            xT33 = ap.tile([EMB + 1, B], f16, name="xT33")
            nc.vector.tensor_copy(xT33[EMB : EMB + 1, :], w["ones_row"][:])
            h0T = fresh("h0T", 512)
            h1T = fresh("h1T", 1024)
            nc.vector.memset(h0T[:].bitcast(f32), 0.0)
            nc.vector.memset(h1T[:].bitcast(f32), 0.0)

            def fresh_gT(tag):
                return hp.tile([128, NCORES * 2 * 128], f16, name=tag, tag=tag)

            def RK2(t, r, k):
                return t[:, (2 * r + k) * 128 : (2 * r + k) * 128 + 128]

            gT = fresh_gT("gT")
            nc.vector.memset(gT[:].bitcast(f32), 0.0)

            # transpose [B, ncols] (ncols multiple of 128 or 128-chunk count)
            # into a fresh SBUF tile [128, ncols] via PSUM
            def transpose_to(tag, src, ncols, pool=None):
                pool = pool or tp_pool
                dstT = pool.tile([128, ncols], f16, name=tag, tag=tag)
                nch = ncols // 128
                for c0 in range(0, nch, 4):
                    cc = min(4, nch - c0)
                    pt = pst.tile([128, 512], f16, name=f"pt_{tag}_{c0}", tag="pt")
                    for c in range(cc):
                        nc.tensor.transpose(
                            pt[:, c * 128 : c * 128 + 128],
                            src[:, (c0 + c) * 128 : (c0 + c + 1) * 128],
                            w["ident"][:],
                        )
                    nc.vector.tensor_copy(
                        dstT[:, c0 * 128 : (c0 + cc) * 128], pt[:, 0 : cc * 128]
                    )
                return dstT

            def stage_and_gather(src, tag, wv):
                # src = [B, 256] slice state -> transposed [128, 256] -> AG
                pt = pst.tile([128, 512], f16, name=f"pt_{tag}{wv}", tag="pt")
                nc.tensor.transpose(pt[:, 0:128], src[:, 0:128], w["ident"][:])
                nc.tensor.transpose(pt[:, 128:256], src[:, 128:256], w["ident"][:])
                stg = sp.tile([128, 256], f16, name=f"stg_{tag}{wv}", tag=f"stg{tag}")
                nc.vector.tensor_copy(stg[:], pt[:, 0:256])
                agin = dp.tile([128, 256], f16, name=f"agin_{tag}{wv}", tag=f"agin{tag}")
                agout = dp.tile(
                    [NCORES * 128, 256], f16,
                    name=f"agout_{tag}{wv}", tag=f"agout{tag}",
                )
                nc.sync.dma_start(agin[:], stg[:])
                if SKIP_COLL:
                    nc.sync.dma_start(agout[0:128, :], agin[:])
                else:
                    nc.gpsimd.collective_compute(
                        "AllGather",
                        mybir.AluOpType.bypass,
                        replica_groups=[list(range(NCORES))],
                        ins=[agin[:]],
                        outs=[agout[:]],
                    )
                return agout

            def unpack(agout, tag):
                t = fresh_gT(tag)
                if SKIP_UNPACK:
                    nc.sync.dma_start(t[:, 0:256], agout[0:128, :])
                    return t
                src3 = agout[:].rearrange("(r p) c -> r p c", r=NCORES, p=128).transpose([1, 0, 2])
                dst3 = t[:].rearrange("p (r c) -> p r c", r=NCORES, c=256)
                nc.sync.dma_start(dst3, src3)
                return t

            def mm_block(pg, pgoff, wcols, a, xT, xchunks, hT, nch,
                         kgx, kgh, G, bias_row):
                """Accumulate one <=512-col output region: bias + x + h parts.
                Region-complete bracket (start on first, stop on last)."""
                out = pg[:, pgoff : pgoff + wcols]
                first = True
                if bias_row is not None:
                    mm(out, w["ones_row"][:], bias_row[0:1, a : a + wcols],
                       start=True, stop=False)
                    first = False
                if xchunks == 0:
                    mm(out, xT, kgx[0 : EMB + 1, a : a + wcols],
                       start=first, stop=False)
                    first = False
                else:
                    for j in range(xchunks):
                        mm(out, xT[:, j * 128 : (j + 1) * 128],
                           kgx[:, j * G + a : j * G + a + wcols],
                           start=first, stop=False)
                        first = False
                for j in range(nch):
                    mm(out, hT[:, j * 128 : (j + 1) * 128],
                       kgh[:, j * G + a : j * G + a + wcols],
                       start=first, stop=(j == nch - 1))
                    first = False

            def gru_full(l, t_idx, xT, xchunks, hT, cin_h, cout, h_t, rh_t,
                         kgx, kcx, kgh, kch, bias_g, bias_c, wv):
                """Replicated full-width GRU step for layer l (0 or 1).
                xT: transposed input (xembT slice for l=0), hT: transposed own
                h. Updates h_t/rh_t in place; returns the new transposed h."""
                nch = cin_h // 128
                G = 2 * cout
                gsb = gp.tile([B, G], f16, name=f"g{l}sb_{wv}", tag=f"g{l}sb")
                for a0 in range(0, G, 1024):
                    W_ = min(1024, G - a0)
                    pg = psb.tile([B, 1024], f32, name=f"pg{l}_{a0}_{wv}", tag="psb")
                    for s0 in range(0, W_, 512):
                        mm_block(pg, s0, min(512, W_ - s0), a0 + s0,
                                 xT, xchunks, hT, nch, kgx, kgh, G, bias_g)
                    nc.scalar.activation(gsb[:, a0 : a0 + W_], pg[:, 0:W_], AF.Sigmoid,
                                         bias=w["maskb"][:, t_idx : t_idx + 1])
                nc.vector.tensor_mul(rh_t[:], gsb[:, 0:cout], h_t[:])
                rhT = transpose_to(f"rh{l}T", rh_t, cout, pool=tpr_pool)
                csb = gp.tile([B, cout], f16, name=f"c{l}sb_{wv}", tag="csb")
                for a0 in range(0, cout, 1024):
                    W_ = min(1024, cout - a0)
                    pc = psb.tile([B, 1024], f32, name=f"pc{l}_{a0}_{wv}", tag="psb")
                    for s0 in range(0, W_, 512):
                        mm_block(pc, s0, min(512, W_ - s0), a0 + s0,
                                 xT, xchunks, rhT, nch, kcx, kch, cout, bias_c)
                    nc.scalar.activation(csb[:, a0 : a0 + W_], pc[:, 0:W_], AF.Tanh)
                tmp = gp.tile([B, cout], f16, name=f"t{l}_{wv}", tag="tmpf")
                nc.vector.tensor_sub(tmp[:], h_t[:], csb[:])
                nc.vector.tensor_mul(tmp[:], gsb[:, cout : 2 * cout], tmp[:])
                nc.vector.tensor_add(h_t[:], tmp[:], csb[:])
                return transpose_to(f"h{l}T", h_t, cout)

            for rep in range(repeat):
              for wv0 in range(n_waves):
                wv = rep * n_waves + wv0
                t0, t1, t2 = wv0, wv0 - 1, wv0 - 2
                last_wave = rep == repeat - 1 and wv0 == n_waves - 1

                # ---------------- L2 gates (sharded) ----------------
                pg2 = ps2.tile([B, 2 * SL2], f32, name=f"pg2_{wv}", tag="pg2")
                g2sb = gp.tile([B, 2 * SL2], f16, name=f"g2sb_{wv}", tag="g2sb")
                if 0 <= t2 < S and not SKIP_L2MM:
                    mm(pg2[:], w["ones_row"][:], w["bg2row"][:], start=True, stop=False)
                    for j in range(8):  # x part: h1T (prev wave)
                        mm(pg2[:], h1T[:, j * 128 : (j + 1) * 128],
                           wt("kg2x", 2 * SL2, j), start=False, stop=False)
                    for j in range(16):  # h part: gathered h2
                        mm(pg2[:], gT[:, j * 128 : (j + 1) * 128],
                           wt("kg2h", 2 * SL2, j), start=False, stop=(j == 15))
                    nc.scalar.activation(g2sb[:], pg2[:], AF.Sigmoid,
                                         bias=w["maskb"][:, t2 : t2 + 1])
                    nc.vector.tensor_mul(rh2[:], g2sb[:, 0:SL2], h2[:])
                ag_r2 = stage_and_gather(rh2, "r", wv)

                # ---------------- L2 cand x-parts (pre-issue) ----------------
                pc2 = ps2.tile([B, SL2], f32, name=f"pc2_{wv}", tag="pc2")
                if 0 <= t2 < S and not SKIP_L2MM:
                    mm(pc2[:], w["ones_row"][:], w["bc2row"][:], start=True, stop=False)
                    for j in range(8):
                        mm(pc2[:], h1T[:, j * 128 : (j + 1) * 128],
                           wt("kc2x", SL2, j), start=False, stop=False)

                # ---------------- L0 replicated chain ----------------
                if t0 < S and not SKIP_L01:
                    ptx = pst.tile([128, 512], f16, name=f"ptx_{wv}", tag="pt")
                    nc.tensor.transpose(
                        ptx[0:EMB, 0:128],
                        w["xembB"][:, t0 * EMB : (t0 + 1) * EMB],
                        w["ident"][:],
                    )
                    nc.vector.tensor_copy(xT33[0:EMB, :], ptx[0:EMB, 0:128])
                    h0T = gru_full(
                        0, t0, xT33[:], 0,
                        h0T, 512, 512, h0, rh0,
                        w["kg0x"], w["kc0x"], w["kg0h"], w["kc0h"],
                        None, None, wv,
                    )

                # ---------------- L2 cand rh-parts + h2 update ----------------
                rT = unpack(ag_r2, "rT")
                if 0 <= t2 < S and not SKIP_L2MM:
                    for j in range(16):
                        mm(pc2[:], rT[:, j * 128 : (j + 1) * 128],
                           wt("kc2h", SL2, j), start=False, stop=(j == 15))
                    c2sb = gp.tile([B, SL2], f16, name=f"c2sb_{wv}", tag="c2sb")
                    nc.scalar.activation(c2sb[:], pc2[:], AF.Tanh)
                    tmp2 = gp.tile([B, SL2], f16, name=f"tmp2_{wv}", tag="tmp2")
                    nc.vector.tensor_sub(tmp2[:], h2[:], c2sb[:])
                    nc.vector.tensor_mul(tmp2[:], g2sb[:, SL2 : 2 * SL2], tmp2[:])
                    nc.vector.tensor_add(h2[:], tmp2[:], c2sb[:])
                if not last_wave:
                    ag_h2 = stage_and_gather(h2, "h", wv)

                # ---------------- L1 replicated chain ----------------
                if 0 <= t1 < S and not SKIP_L01:
                    h1T = gru_full(
                        1, t1, h0T_prev, 4,
                        h1T, 1024, 1024, h1, rh1,
                        w["kg1x"], w["kc1x"], w["kg1h"], w["kc1h"],
                        w["bg1row"], w["bc1row"], wv,
                    )

                h0T_prev = h0T
                if not last_wave:
                    gT = unpack(ag_h2, "gT")

            # ---------------- final projection ----------------
            # lhsT chunks: h1T (8) + h2T (2) + h0T (4) = 14
            ptf = pst.tile([128, 512], f16, name="ptf", tag="pt")
            nc.tensor.transpose(ptf[:, 0:128], h2[:, 0:128], w["ident"][:])
            nc.tensor.transpose(ptf[:, 128:256], h2[:, 128:256], w["ident"][:])
            h2T = fresh("h2T", 256)
            nc.vector.tensor_copy(h2T[:], ptf[:, 0:256])
            pz = ps2.tile([B, LATENT], f32, name="pz", tag="pg2")
            mm(pz[:], w["ones_row"][:], w["benc_row"][:], start=True, stop=False)
            chunks = [(h1T, j) for j in range(8)] + [(h2T, j) for j in range(2)] + [(h0T, j) for j in range(4)]
            for k, (tl, j) in enumerate(chunks):
                mm(pz[:], tl[:, j * 128 : (j + 1) * 128],
                   wt("wenc", LATENT, k), start=False, stop=(k == 13))
            zsb = gp.tile([B, LATENT], f32, name="zsb", tag="g2sb")
            nc.vector.tensor_copy(zsb[:], pz[:])
            arin = dp.tile([B, LATENT], f32, name="arin")
            arout = dp.tile([B, LATENT], f32, name="arout", addr_space="Shared")
            nc.sync.dma_start(arin[:], zsb[:])
            if SKIP_COLL:
                nc.sync.dma_start(arout[:], arin[:])
            else:
                nc.gpsimd.collective_compute(
                    "AllReduce",
                    mybir.AluOpType.add,
                    replica_groups=[list(range(NCORES))],
                    ins=[arin[:]],
                    outs=[arout[:]],
                )
            zfull = gp.tile([B, LATENT], f32, name="zfull", tag="g1sb")
            nc.sync.dma_start(zfull[:], arout[:])
            ofin = gp.tile([B, LATENT], f32, name="ofin", tag="g0sb")
            nc.scalar.activation(ofin[:], zfull[:], AF.Tanh)
            nc.sync.dma_start(out_d[:], ofin[:])

    nc.compile()
    return nc


_NC_CACHE = {}


def kernel(**inputs) -> np.ndarray:
    from concourse import bass_utils

    if "nc" not in _NC_CACHE:
        _NC_CACHE["nc"] = build()
    nc = _NC_CACHE["nc"]
    in_maps = prepack(inputs)
    res = bass_utils.run_bass_kernel_spmd(nc, in_maps, core_ids=list(range(NCORES)))
    return np.asarray(res.results[0]["out"], np.float32)
